# revision 6
# baseline (speedup 1.0000x reference)
"""GQA attention kernel for 8 Trainium2 NeuronCores — v2.

Problem: B=2, S=2048, D=2048, 16 q-heads / 4 kv-heads (GQA), head_dim=128,
causal mask, RoPE over the full hidden dim, scale 1/sqrt(D), output proj.

Sharding: core c = 4*b + g handles batch b and head-group g: q-heads
4g..4g+3 sharing kv-head g.  Output projection partials summed on host.

v2 changes vs baseline:
  - bf16 operands everywhere (DMA/SBUF halved, full PE rate at any free
    size -> finer causal trimming).
  - Q/K projections in fp8e4 with DoubleRow perf mode (2 contraction
    tiles per instruction, 0.5 cyc/row).  Weights pre-scaled by 16 to
    stay in e4m3 normal range; folded back via the exp scale (1/256).
  - RoPE add performed in PSUM: qt = I@(q*ra) + pswap@(q*rbs) where
    rbs is the row-swapped sin table (host precomputed), removing the
    vector-engine adds.
  - Causal mask applied post-exp as a binary multiply on the Pool
    engine (only the [128,128] triangle of each diagonal block),
    keeping DVE off the PE->ACT critical chain.
  - Softmax denominator reciprocal broadcast via gpsimd
    partition_broadcast (no PSUM bank, no PE matmul).
  - Phases fused: attention runs qc-outer/i-inner; the output
    projection for q-chunk qc is emitted after attn(qc+1, i=0) so PE
    bubbles are filled and normalization latency is hidden.
"""

import sys

sys.path.insert(0, "/opt/trn_rl_repo")

from contextlib import ExitStack

import numpy as np
import ml_dtypes

import concourse.bass as bass
import concourse.tile as tile
from concourse import bacc, mybir
from concourse.bass_utils import run_bass_kernel_spmd
from concourse.masks import make_identity

B, S, D = 2, 2048, 2048
NH, NG = 16, 4
KVH = NH // NG  # 4
HD = D // NH  # 128
HPC = 4  # q heads per core
ROPE_THETA = 10000.0
W8SCALE = 16.0  # fp8 weight pre-scale (Q and K)
EXP_SCALE = float(1.0 / (np.sqrt(np.float64(D)) * W8SCALE * W8SCALE))

F32 = mybir.dt.float32
BF16 = mybir.dt.bfloat16
FP8 = mybir.dt.float8e4
DR = mybir.MatmulPerfMode.DoubleRow

N_DT = D // 128  # 16 contraction tiles
N_SC = S // 512  # 4 seq chunks
N_SB = S // 128  # 16 seq blocks


def build_kernel_body(ctx: ExitStack, tc: tile.TileContext, outT, ins):
    nc = tc.nc
    (xT, x8, wq8d, wk8d, wvTd, woTd, ropeA, ropeBs, trid, pswapd,
     oneskd) = ins

    # ---------------- persistent tiles ----------------
    persist = ctx.enter_context(tc.tile_pool(name="persist", bufs=1))
    qkt = persist.tile([128, HPC, 2, S], BF16)  # [:, h, 0] = qt, [:, h, 1] = kt
    vsb = persist.tile([128, N_SB, 128], BF16)  # V s-major
    at = persist.tile([128, HPC, S], BF16)  # attention out, hd-major
    wq_sb = persist.tile([128, N_DT, 512], FP8)
    wk_sb = persist.tile([128, N_DT, 128], FP8)
    wv_sb = persist.tile([128, N_DT, 128], BF16)
    wo_sb = persist.tile([128, HPC, S], BF16)
    psw = persist.tile([128, 128], BF16)
    tri = persist.tile([128, 128], BF16)
    onesk = persist.tile([128, 1], BF16)
    ident = persist.tile([128, 128], BF16)
    zbias = persist.tile([128, 1], F32)
    nc.gpsimd.memset(zbias[:], 0.0)
    make_identity(nc, ident[:])

    # weight/const DMAs on the Pool queue (cheap issue), in need order
    for dt in range(N_DT):
        nc.gpsimd.dma_start(wk_sb[:, dt, :], wk8d[128 * dt:128 * (dt + 1), :])
    for dt in range(N_DT):
        nc.gpsimd.dma_start(wv_sb[:, dt, :], wvTd[128 * dt:128 * (dt + 1), :])
    nc.gpsimd.dma_start(psw[:], pswapd[:])
    nc.gpsimd.dma_start(tri[:], trid[:])
    nc.gpsimd.dma_start(onesk[:], oneskd[:])

    # ---------------- phase 1: projections + RoPE ----------------
    with tc.tile_pool(name="xc", bufs=2) as xcpool, \
         tc.tile_pool(name="x8c", bufs=2) as x8pool, \
         tc.tile_pool(name="rope", bufs=2) as rpool, \
         tc.tile_pool(name="kraw", bufs=2) as kpool, \
         tc.tile_pool(name="vtr", bufs=2) as vpool, \
         tc.tile_pool(name="t12", bufs=6) as tpool, \
         tc.tile_pool(name="pkv", bufs=1, space="PSUM") as pkv, \
         tc.tile_pool(name="pq", bufs=3, space="PSUM") as pq, \
         tc.tile_pool(name="pqk", bufs=2, space="PSUM") as pqk:

        first_wq = True
        for c in range(N_SC):
            cs = slice(512 * c, 512 * (c + 1))
            xc = xcpool.tile([128, N_DT, 512], BF16, name="xc_t")
            for dt in range(N_DT):
                nc.sync.dma_start(xc[:, dt, :], xT[128 * dt:128 * (dt + 1), cs])
            x8c = x8pool.tile([128, N_DT, 512], FP8, name="x8_t")
            for dt in range(N_DT):
                nc.gpsimd.dma_start(x8c[:, dt, :],
                                    x8[128 * dt:128 * (dt + 1), cs])
            if first_wq:
                # wq after wk/wv/x8-c0 so K can start earliest
                for dt in range(N_DT):
                    nc.gpsimd.dma_start(wq_sb[:, dt, :],
                                        wq8d[128 * dt:128 * (dt + 1), :])
                first_wq = False
            ra = rpool.tile([128, HPC, 512], BF16, name="ra_t")
            rb = rpool.tile([128, HPC, 512], BF16, name="rb_t")
            for i in range(HPC):
                nc.gpsimd.dma_start(ra[:, i, :],
                                    ropeA[128 * i:128 * (i + 1), cs])
                nc.gpsimd.dma_start(rb[:, i, :],
                                    ropeBs[128 * i:128 * (i + 1), cs])

            # K projection (fp8 DoubleRow); K/V/vT share one psum bank
            k_ps = pkv.tile([128, 512], F32, name="kv_t")
            for t in range(8):
                nc.tensor.matmul(k_ps[:], wk_sb[:, 2 * t:2 * t + 2, :],
                                 x8c[:, 2 * t:2 * t + 2, :],
                                 start=(t == 0), stop=(t == 7), perf_mode=DR)
            # Q projections for heads 0..2 (3 psum bufs)
            q_ps = []
            for i in range(3):
                qp = pq.tile([128, 512], F32, name="q_t")
                for t in range(8):
                    nc.tensor.matmul(
                        qp[:], wq_sb[:, 2 * t:2 * t + 2, 128 * i:128 * (i + 1)],
                        x8c[:, 2 * t:2 * t + 2, :],
                        start=(t == 0), stop=(t == 7), perf_mode=DR)
                q_ps.append(qp)
            kraw = kpool.tile([128, 512], BF16, name="kraw_t")
            nc.scalar.copy(kraw[:], k_ps[:])
            # V projection (bf16), same bank after kraw eviction
            v_ps = pkv.tile([128, 512], F32, name="kv_t")
            for dt in range(N_DT):
                nc.tensor.matmul(v_ps[:], wv_sb[:, dt, :], xc[:, dt, :],
                                 start=(dt == 0), stop=(dt == N_DT - 1))
            # Q head 3 (reuses first pq buf after head-0 rope muls read it)
            qp = pq.tile([128, 512], F32, name="q_t")
            for t in range(8):
                nc.tensor.matmul(
                    qp[:], wq_sb[:, 2 * t:2 * t + 2, 384:512],
                    x8c[:, 2 * t:2 * t + 2, :],
                    start=(t == 0), stop=(t == 7), perf_mode=DR)
            q_ps.append(qp)
            vtr = vpool.tile([128, 512], BF16, name="vtr_t")
            nc.scalar.copy(vtr[:], v_ps[:])
            # V transpose to s-major, same bank again (bf16 psum tile,
            # padded to the same slot byte size)
            vt_ps = pkv.tile([128, 1024], BF16, name="kv_t")
            for j in range(4):
                nc.tensor.transpose(vt_ps[:, 128 * j:128 * (j + 1)],
                                    vtr[:, 128 * j:128 * (j + 1)], ident[:])
            nc.scalar.copy(vsb[:, 4 * c:4 * c + 4, :], vt_ps[:, 0:512])

            # RoPE per head: qt = I@(q*ra) + psw@(q*rbs), same for k
            for i in range(HPC):
                t1q = tpool.tile([128, 512], BF16, name="t1q")
                nc.vector.tensor_mul(t1q[:], q_ps[i][:], ra[:, i, :])
                t2q = tpool.tile([128, 512], BF16, name="t2q")
                nc.vector.tensor_mul(t2q[:], q_ps[i][:], rb[:, i, :])
                t1k = tpool.tile([128, 512], BF16, name="t1k")
                nc.vector.tensor_mul(t1k[:], kraw[:], ra[:, i, :])
                t2k = tpool.tile([128, 512], BF16, name="t2k")
                nc.gpsimd.tensor_mul(t2k[:], kraw[:], rb[:, i, :])
                qk_ps = pqk.tile([128, 2, 512], F32, name="qk_t")
                nc.tensor.matmul(qk_ps[:, 0, :], ident[:], t1q[:],
                                 start=True, stop=False)
                nc.tensor.matmul(qk_ps[:, 0, :], psw[:], t2q[:],
                                 start=False, stop=True)
                nc.tensor.matmul(qk_ps[:, 1, :], ident[:], t1k[:],
                                 start=True, stop=False)
                nc.tensor.matmul(qk_ps[:, 1, :], psw[:], t2k[:],
                                 start=False, stop=True)
                nc.scalar.copy(qkt[:, i, :, cs], qk_ps[:])

    # wo on sync queue after the x chunks
    for h in range(HPC):
        nc.sync.dma_start(wo_sb[:, h, :], woTd[128 * h:128 * (h + 1), :])

    # ---------------- phase 2+3: attention + output projection ----------
    with tc.tile_pool(name="stp", bufs=2, space="PSUM") as stp, \
         tc.tile_pool(name="ovp", bufs=2, space="PSUM") as ovp, \
         tc.tile_pool(name="sump", bufs=2, space="PSUM") as sump, \
         tc.tile_pool(name="ptp", bufs=4) as ptp, \
         tc.tile_pool(name="rcpp", bufs=2) as rcpp, \
         tc.tile_pool(name="bcp", bufs=2) as bcp, \
         tc.tile_pool(name="osp", bufs=3) as osp:

        pending_norm = None

        def emit_p3(qc):
            for jp in range(8):
                op = stp.tile([128, 2, 512], F32, name="st_t")
                for s_ in range(2):
                    jb = 2 * jp + s_
                    for h in range(HPC):
                        nc.tensor.matmul(
                            op[:, s_, :],
                            wo_sb[:, h, 128 * jb:128 * (jb + 1)],
                            at[:, h, 512 * qc:512 * (qc + 1)],
                            start=(h == 0), stop=(h == HPC - 1))
                ob = osp.tile([128, 2, 512], BF16, name="ob_t")
                with nc.allow_low_precision(reason="bf16 partial out"):
                    nc.vector.tensor_copy(ob[:], op[:])
                for s_ in range(2):
                    jb = 2 * jp + s_
                    nc.sync.dma_start(
                        outT[128 * jb:128 * (jb + 1), 512 * qc:512 * (qc + 1)],
                        ob[:, s_, :])

        for qc in range(N_SC):
            for i in range(HPC):
                ov = ovp.tile([128, 512], F32, name="ov_t")
                sm = sump.tile([1, 512], F32, name="sm_t")
                qs = 512 * qc

                # work units: off-diagonal pairs, then 4 diagonal blocks
                units = [("pair", p) for p in range(2 * qc)]
                units += [("diag", j) for j in range(4)]
                nu = len(units)

                def emit_pv(u, pt, first, last, i=i, qc=qc, ov=ov, sm=sm):
                    kind, idx = u
                    if kind == "pair":
                        for s_ in range(2):
                            kb = 2 * idx + s_
                            st_ = first and s_ == 0
                            nc.tensor.matmul(ov[:], vsb[:, kb, :],
                                             pt[:, s_, :],
                                             start=st_, stop=last and s_ == 1)
                            nc.tensor.matmul(sm[:], onesk[:], pt[:, s_, :],
                                             start=st_, stop=last and s_ == 1)
                    else:
                        j = idx
                        kb = 4 * qc + j
                        qo = 128 * j
                        nc.tensor.matmul(ov[:, qo:], vsb[:, kb, :],
                                         pt[:, qo:], start=first, stop=last)
                        nc.tensor.matmul(sm[:, qo:], onesk[:], pt[:, qo:],
                                         start=first, stop=last)

                prev = None
                for ui, u in enumerate(units):
                    kind, idx = u
                    if kind == "pair":
                        st2 = stp.tile([128, 2, 512], F32, name="st_t")
                        for s_ in range(2):
                            kb = 2 * idx + s_
                            nc.tensor.matmul(
                                st2[:, s_, :],
                                qkt[:, i, 1, 128 * kb:128 * (kb + 1)],
                                qkt[:, i, 0, qs:qs + 512],
                                start=True, stop=True)
                        pt = ptp.tile([128, 2, 512], BF16, name="pt_t")
                        nc.scalar.activation(
                            pt[:], st2[:],
                            mybir.ActivationFunctionType.Exp,
                            bias=zbias[:], scale=EXP_SCALE)
                    else:
                        j = idx
                        kb = 4 * qc + j
                        qo = 128 * j
                        st2 = stp.tile([128, 2, 512], F32, name="st_t")
                        nc.tensor.matmul(
                            st2[:, 0, qo:],
                            qkt[:, i, 1, 128 * kb:128 * (kb + 1)],
                            qkt[:, i, 0, qs + qo:qs + 512],
                            start=True, stop=True)
                        pt = ptp.tile([128, 2, 512], BF16, name="pt_t")
                        nc.scalar.activation(
                            pt[:, 0, qo:], st2[:, 0, qo:],
                            mybir.ActivationFunctionType.Exp,
                            bias=zbias[:], scale=EXP_SCALE)
                        # causal triangle: binary mask multiply (Pool)
                        nc.gpsimd.tensor_mul(pt[:, 0, qo:qo + 128],
                                             pt[:, 0, qo:qo + 128], tri[:])
                        pt = pt[:, 0, :]
                    if ui == 1 and pending_norm is not None:
                        pending_norm()
                        pending_norm = None
                    if prev is not None:
                        emit_pv(prev[0], prev[1], prev[2], False)
                    prev = (u, pt, ui == 0)
                emit_pv(prev[0], prev[1], prev[2], True)
                if nu == 4 and pending_norm is not None:
                    # qc=0 groups are short; make sure norms drain
                    pending_norm()
                    pending_norm = None

                def norm(i=i, qc=qc, ov=ov, sm=sm):
                    rcp = rcpp.tile([1, 512], F32, name="rcp_t")
                    nc.vector.reciprocal(rcp[:], sm[:])
                    bc = bcp.tile([128, 512], F32, name="bc_t")
                    nc.gpsimd.partition_broadcast(bc[:], rcp[:])
                    with nc.allow_low_precision(reason="bf16 attn out"):
                        nc.vector.tensor_mul(
                            at[:, i, 512 * qc:512 * (qc + 1)], ov[:], bc[:])

                pending_norm = norm
                if qc > 0 and i == 0:
                    emit_p3(qc - 1)

        pending_norm()
        emit_p3(N_SC - 1)


_NC_CACHE = None


def get_nc():
    global _NC_CACHE
    if _NC_CACHE is not None:
        return _NC_CACHE
    nc = bacc.Bacc("TRN2", target_bir_lowering=False, debug=False,
                   num_devices=8)
    xT = nc.dram_tensor("xT", [D, S], BF16, kind="ExternalInput").ap()
    x8 = nc.dram_tensor("x8", [D, S], FP8, kind="ExternalInput").ap()
    wq8 = nc.dram_tensor("wq8", [D, 512], FP8, kind="ExternalInput").ap()
    wk8 = nc.dram_tensor("wk8", [D, 128], FP8, kind="ExternalInput").ap()
    wvT = nc.dram_tensor("wvT", [D, 128], BF16, kind="ExternalInput").ap()
    woT = nc.dram_tensor("woT", [512, S], BF16, kind="ExternalInput").ap()
    ropeA = nc.dram_tensor("ropeA", [512, S], BF16, kind="ExternalInput").ap()
    ropeBs = nc.dram_tensor("ropeBs", [512, S], BF16,
                            kind="ExternalInput").ap()
    trid = nc.dram_tensor("trid", [128, 128], BF16, kind="ExternalInput").ap()
    pswap = nc.dram_tensor("pswap", [128, 128], BF16,
                           kind="ExternalInput").ap()
    onesd = nc.dram_tensor("onesd", [128, 1], BF16, kind="ExternalInput").ap()
    outT = nc.dram_tensor("outT", [D, S], BF16, kind="ExternalOutput").ap()

    with tile.TileContext(nc) as tc, ExitStack() as ctx:
        build_kernel_body(ctx, tc, outT,
                          (xT, x8, wq8, wk8, wvT, woT, ropeA, ropeBs, trid,
                           pswap, onesd))
    nc.compile()
    _NC_CACHE = nc
    return nc


def host_inputs(x, Wq, Wk, Wv, Wo):
    """Per-core input dicts (core c = 4*b + g)."""
    bf16 = ml_dtypes.bfloat16
    f8 = ml_dtypes.float8_e4m3
    x = np.asarray(x, np.float32)
    Wq = np.asarray(Wq, np.float32)
    Wk = np.asarray(Wk, np.float32)
    Wv = np.asarray(Wv, np.float32)
    Wo = np.asarray(Wo, np.float32)

    freqs = 1.0 / (ROPE_THETA ** (np.arange(0, D, 2, dtype=np.float32) / D))
    ang = np.arange(S, dtype=np.float32)[:, None] * freqs[None, :]  # [S, D/2]
    cos = np.cos(ang).astype(np.float32)
    sin = np.sin(ang).astype(np.float32)

    idx = np.arange(128)
    tri = (idx[:, None] <= idx[None, :]).astype(bf16)  # [kp, t] valid
    pswap = np.zeros((128, 128), np.float32)
    pswap[idx, idx ^ 1] = 1.0
    sgn = np.where(idx % 2 == 0, -1.0, 1.0).astype(np.float32)

    xTb = [np.ascontiguousarray(x[b].T) for b in range(B)]

    in_maps = []
    for c in range(8):
        b, g = divmod(c, 4)
        ra = np.empty((512, S), np.float32)
        rbs = np.empty((512, S), np.float32)
        for i in range(HPC):
            fidx = 256 * g + 64 * i + (idx // 2)
            ra[128 * i:128 * (i + 1)] = cos[:, fidx].T
            rb_i = (sin[:, fidx] * sgn[None, :]).T  # [128, S]
            rbs[128 * i:128 * (i + 1)] = rb_i[idx ^ 1]
        in_maps.append({
            "xT": xTb[b].astype(bf16),
            "x8": xTb[b].astype(f8),
            "wq8": np.ascontiguousarray(
                (W8SCALE * Wq[512 * g:512 * (g + 1)]).T).astype(f8),
            "wk8": np.ascontiguousarray(
                (W8SCALE * Wk[128 * g:128 * (g + 1)]).T).astype(f8),
            "wvT": np.ascontiguousarray(
                Wv[128 * g:128 * (g + 1)].T).astype(bf16),
            "woT": np.ascontiguousarray(
                Wo[:, 512 * g:512 * (g + 1)].T).astype(bf16),
            "ropeA": ra.astype(bf16),
            "ropeBs": rbs.astype(bf16),
            "trid": tri,
            "pswap": pswap.astype(bf16),
            "onesd": np.ones((128, 1), bf16),
        })
    return in_maps


def kernel(x, Wq, Wk, Wv, Wo, mask, _trace=False):
    in_maps = host_inputs(x, Wq, Wk, Wv, Wo)
    nc = get_nc()
    res = run_bass_kernel_spmd(nc, in_maps, list(range(8)), trace=_trace)
    outs = [np.asarray(res.results[c]["outT"], np.float32) for c in range(8)]
    out = np.stack([
        (outs[4 * b + 0] + outs[4 * b + 1] + outs[4 * b + 2]
         + outs[4 * b + 3]).T
        for b in range(B)
    ]).astype(np.float32)
    if _trace:
        kernel.last_result = res
    return out


# revision 14
# speedup vs baseline: 1.0943x; 1.0943x over previous
"""GQA attention kernel for 8 Trainium2 NeuronCores — v2.

Problem: B=2, S=2048, D=2048, 16 q-heads / 4 kv-heads (GQA), head_dim=128,
causal mask, RoPE over the full hidden dim, scale 1/sqrt(D), output proj.

Sharding: core c = 4*b + g handles batch b and head-group g: q-heads
4g..4g+3 sharing kv-head g.  Output projection partials summed on host.

v2 changes vs baseline:
  - bf16 operands everywhere (DMA/SBUF halved, full PE rate at any free
    size -> finer causal trimming).
  - Q/K projections in fp8e4 with DoubleRow perf mode (2 contraction
    tiles per instruction, 0.5 cyc/row).  Weights pre-scaled by 16 to
    stay in e4m3 normal range; folded back via the exp scale (1/256).
  - RoPE add performed in PSUM: qt = I@(q*ra) + pswap@(q*rbs) where
    rbs is the row-swapped sin table (host precomputed), removing the
    vector-engine adds.
  - Causal mask applied post-exp as a binary multiply on the Pool
    engine (only the [128,128] triangle of each diagonal block),
    keeping DVE off the PE->ACT critical chain.
  - Softmax denominator reciprocal broadcast via gpsimd
    partition_broadcast (no PSUM bank, no PE matmul).
  - Phases fused: attention runs qc-outer/i-inner; the output
    projection for q-chunk qc is emitted after attn(qc+1, i=0) so PE
    bubbles are filled and normalization latency is hidden.
"""

import sys

sys.path.insert(0, "/opt/trn_rl_repo")

from contextlib import ExitStack

import numpy as np
import ml_dtypes

import concourse.bass as bass
import concourse.tile as tile
from concourse import bacc, mybir
from concourse.bass_utils import run_bass_kernel_spmd
from concourse.masks import make_identity

B, S, D = 2, 2048, 2048
NH, NG = 16, 4
KVH = NH // NG  # 4
HD = D // NH  # 128
HPC = 4  # q heads per core
ROPE_THETA = 10000.0
W8SCALE = 16.0  # fp8 weight pre-scale (Q and K)
EXP_SCALE = float(1.0 / (np.sqrt(np.float64(D)) * W8SCALE * W8SCALE))

F32 = mybir.dt.float32
BF16 = mybir.dt.bfloat16
FP8 = mybir.dt.float8e4
DR = mybir.MatmulPerfMode.DoubleRow

N_DT = D // 128  # 16 contraction tiles
N_SC = S // 512  # 4 seq chunks
N_SB = S // 128  # 16 seq blocks


def build_kernel_body(ctx: ExitStack, tc: tile.TileContext, outT, ins):
    nc = tc.nc
    (xT, x8, wq8d, wk8d, wvTd, woTd, ropeA, ropeBs, trid, pswapd,
     oneskd) = ins

    # ---------------- persistent tiles ----------------
    persist = ctx.enter_context(tc.tile_pool(name="persist", bufs=1))
    qkt = persist.tile([128, HPC, 2, S], BF16)  # [:, h, 0] = qt, [:, h, 1] = kt
    vsb = persist.tile([128, N_SB, 128], BF16)  # V s-major
    at = persist.tile([128, HPC, S], BF16)  # attention out, hd-major
    wq_sb = persist.tile([128, N_DT, 512], FP8)
    wk_sb = persist.tile([128, N_DT, 128], FP8)
    wv_sb = persist.tile([128, N_DT, 128], BF16)
    wo_sb = persist.tile([128, HPC, S], BF16)
    psw = persist.tile([128, 128], BF16)
    tri = persist.tile([128, 128], BF16)
    onesk = persist.tile([128, 1], BF16)
    ident = persist.tile([128, 128], BF16)
    zbias = persist.tile([128, 1], F32)
    nc.gpsimd.memset(zbias[:], 0.0)
    make_identity(nc, ident[:])

    # weight/const DMAs (one DMA per tensor; p-major packed on host)
    nc.gpsimd.dma_start(wk_sb[:], wk8d[:])
    nc.gpsimd.dma_start(wv_sb[:], wvTd[:])
    nc.gpsimd.dma_start(psw[:], pswapd[:])
    nc.gpsimd.dma_start(tri[:], trid[:])
    nc.gpsimd.dma_start(onesk[:], oneskd[:])

    # ---------------- phase 1: projections + RoPE ----------------
    with tc.tile_pool(name="xc", bufs=2) as xcpool, \
         tc.tile_pool(name="x8c", bufs=2) as x8pool, \
         tc.tile_pool(name="rope", bufs=2) as rpool, \
         tc.tile_pool(name="kraw", bufs=2) as kpool, \
         tc.tile_pool(name="vtr", bufs=2) as vpool, \
         tc.tile_pool(name="t12", bufs=6) as tpool, \
         tc.tile_pool(name="pkv", bufs=1, space="PSUM") as pkv, \
         tc.tile_pool(name="pq", bufs=3, space="PSUM") as pq, \
         tc.tile_pool(name="pqk", bufs=2, space="PSUM") as pqk:

        first_wq = True
        for c in range(N_SC):
            cs = slice(512 * c, 512 * (c + 1))
            xc = xcpool.tile([128, N_DT, 512], BF16, name="xc_t")
            nc.sync.dma_start(xc[:], xT[:, :, c, :])
            x8c = x8pool.tile([128, N_DT, 512], FP8, name="x8_t")
            nc.gpsimd.dma_start(x8c[:], x8[:, :, c, :])
            if first_wq:
                # wq after wk/wv/x8-c0 so K can start earliest
                nc.gpsimd.dma_start(wq_sb[:], wq8d[:])
                first_wq = False
            ra = rpool.tile([128, HPC, 512], BF16, name="ra_t")
            rb = rpool.tile([128, HPC, 512], BF16, name="rb_t")
            nc.scalar.dma_start(ra[:], ropeA[:, :, c, :])
            nc.scalar.dma_start(rb[:], ropeBs[:, :, c, :])

            # K projection (fp8 DoubleRow); K/V/vT share one psum bank
            k_ps = pkv.tile([128, 512], F32, name="kv_t")
            for t in range(8):
                nc.tensor.matmul(k_ps[:], wk_sb[:, 2 * t:2 * t + 2, :],
                                 x8c[:, 2 * t:2 * t + 2, :],
                                 start=(t == 0), stop=(t == 7), perf_mode=DR)
            # Q projections for heads 0..2 (3 psum bufs)
            q_ps = []
            for i in range(3):
                qp = pq.tile([128, 512], F32, name="q_t")
                for t in range(8):
                    nc.tensor.matmul(
                        qp[:], wq_sb[:, 2 * t:2 * t + 2, 128 * i:128 * (i + 1)],
                        x8c[:, 2 * t:2 * t + 2, :],
                        start=(t == 0), stop=(t == 7), perf_mode=DR)
                q_ps.append(qp)
            kraw = kpool.tile([128, 512], BF16, name="kraw_t")
            nc.scalar.copy(kraw[:], k_ps[:])
            # V projection (bf16), same bank after kraw eviction
            v_ps = pkv.tile([128, 512], F32, name="kv_t")
            for dt in range(N_DT):
                nc.tensor.matmul(v_ps[:], wv_sb[:, dt, :], xc[:, dt, :],
                                 start=(dt == 0), stop=(dt == N_DT - 1))
            # Q head 3 (reuses first pq buf after head-0 rope muls read it)
            qp = pq.tile([128, 512], F32, name="q_t")
            for t in range(8):
                nc.tensor.matmul(
                    qp[:], wq_sb[:, 2 * t:2 * t + 2, 384:512],
                    x8c[:, 2 * t:2 * t + 2, :],
                    start=(t == 0), stop=(t == 7), perf_mode=DR)
            q_ps.append(qp)
            vtr = vpool.tile([128, 512], BF16, name="vtr_t")
            nc.scalar.copy(vtr[:], v_ps[:])
            # V transpose to s-major, same bank again (bf16 psum tile,
            # padded to the same slot byte size)
            vt_ps = pkv.tile([128, 1024], BF16, name="kv_t")
            for j in range(4):
                nc.tensor.transpose(vt_ps[:, 128 * j:128 * (j + 1)],
                                    vtr[:, 128 * j:128 * (j + 1)], ident[:])
            nc.scalar.copy(vsb[:, 4 * c:4 * c + 4, :], vt_ps[:, 0:512])

            # RoPE per head: qt = I@(q*ra) + psw@(q*rbs), same for k
            for i in range(HPC):
                t1q = tpool.tile([128, 512], BF16, name="t1q")
                nc.vector.tensor_mul(t1q[:], q_ps[i][:], ra[:, i, :])
                t2q = tpool.tile([128, 512], BF16, name="t2q")
                nc.vector.tensor_mul(t2q[:], q_ps[i][:], rb[:, i, :])
                t1k = tpool.tile([128, 512], BF16, name="t1k")
                nc.vector.tensor_mul(t1k[:], kraw[:], ra[:, i, :])
                t2k = tpool.tile([128, 512], BF16, name="t2k")
                nc.gpsimd.tensor_mul(t2k[:], kraw[:], rb[:, i, :])
                qk_ps = pqk.tile([128, 2, 512], F32, name="qk_t")
                nc.tensor.matmul(qk_ps[:, 0, :], ident[:], t1q[:],
                                 start=True, stop=False)
                nc.tensor.matmul(qk_ps[:, 0, :], psw[:], t2q[:],
                                 start=False, stop=True)
                nc.tensor.matmul(qk_ps[:, 1, :], ident[:], t1k[:],
                                 start=True, stop=False)
                nc.tensor.matmul(qk_ps[:, 1, :], psw[:], t2k[:],
                                 start=False, stop=True)
                nc.scalar.copy(qkt[:, i, :, cs], qk_ps[:])

    # wo on sync queue after the x chunks
    nc.sync.dma_start(wo_sb[:], woTd[:])

    # ---------------- phase 2+3: attention + output projection ----------
    with tc.tile_pool(name="stp", bufs=2, space="PSUM") as stp, \
         tc.tile_pool(name="ovp", bufs=2, space="PSUM") as ovp, \
         tc.tile_pool(name="sump", bufs=2, space="PSUM") as sump, \
         tc.tile_pool(name="ptp", bufs=4) as ptp, \
         tc.tile_pool(name="rcpp", bufs=2) as rcpp, \
         tc.tile_pool(name="bcp", bufs=2) as bcp, \
         tc.tile_pool(name="osp", bufs=3) as osp:

        pending_norm = None

        def emit_p3(qc):
            for jp in range(8):
                op = stp.tile([128, 2, 512], F32, name="st_t")
                for s_ in range(2):
                    jb = 2 * jp + s_
                    for h in range(HPC):
                        nc.tensor.matmul(
                            op[:, s_, :],
                            wo_sb[:, h, 128 * jb:128 * (jb + 1)],
                            at[:, h, 512 * qc:512 * (qc + 1)],
                            start=(h == 0), stop=(h == HPC - 1))
                ob = osp.tile([128, 2, 512], BF16, name="ob_t")
                with nc.allow_low_precision(reason="bf16 partial out"):
                    nc.vector.tensor_copy(ob[:], op[:])
                nc.sync.dma_start(
                    outT[:, 2 * jp:2 * jp + 2, 512 * qc:512 * (qc + 1)],
                    ob[:])

        for qc in range(N_SC):
            for i in range(HPC):
                ov = ovp.tile([128, 512], F32, name="ov_t")
                sm = sump.tile([1, 512], F32, name="sm_t")
                qs = 512 * qc

                # work units: off-diagonal pairs, then 4 diagonal blocks
                units = [("pair", p) for p in range(2 * qc)]
                units += [("diag", j) for j in range(4)]
                nu = len(units)

                def emit_pv(u, pt, first, last, i=i, qc=qc, ov=ov, sm=sm):
                    kind, idx = u
                    if kind == "pair":
                        for s_ in range(2):
                            kb = 2 * idx + s_
                            st_ = first and s_ == 0
                            nc.tensor.matmul(ov[:], vsb[:, kb, :],
                                             pt[:, s_, :],
                                             start=st_, stop=last and s_ == 1)
                            nc.tensor.matmul(sm[:], onesk[:], pt[:, s_, :],
                                             start=st_, stop=last and s_ == 1)
                    else:
                        j = idx
                        kb = 4 * qc + j
                        qo = 128 * j
                        nc.tensor.matmul(ov[:, qo:], vsb[:, kb, :],
                                         pt[:, qo:], start=first, stop=last)
                        nc.tensor.matmul(sm[:, qo:], onesk[:], pt[:, qo:],
                                         start=first, stop=last)

                prev = None
                for ui, u in enumerate(units):
                    kind, idx = u
                    if kind == "pair":
                        st2 = stp.tile([128, 2, 512], F32, name="st_t")
                        for s_ in range(2):
                            kb = 2 * idx + s_
                            nc.tensor.matmul(
                                st2[:, s_, :],
                                qkt[:, i, 1, 128 * kb:128 * (kb + 1)],
                                qkt[:, i, 0, qs:qs + 512],
                                start=True, stop=True)
                        pt = ptp.tile([128, 2, 512], BF16, name="pt_t")
                        nc.scalar.activation(
                            pt[:], st2[:],
                            mybir.ActivationFunctionType.Exp,
                            bias=zbias[:], scale=EXP_SCALE)
                    else:
                        j = idx
                        kb = 4 * qc + j
                        qo = 128 * j
                        st2 = stp.tile([128, 2, 512], F32, name="st_t")
                        nc.tensor.matmul(
                            st2[:, 0, qo:],
                            qkt[:, i, 1, 128 * kb:128 * (kb + 1)],
                            qkt[:, i, 0, qs + qo:qs + 512],
                            start=True, stop=True)
                        pt = ptp.tile([128, 2, 512], BF16, name="pt_t")
                        nc.scalar.activation(
                            pt[:, 0, qo:], st2[:, 0, qo:],
                            mybir.ActivationFunctionType.Exp,
                            bias=zbias[:], scale=EXP_SCALE)
                        # causal triangle: binary mask multiply (Pool)
                        nc.gpsimd.tensor_mul(pt[:, 0, qo:qo + 128],
                                             pt[:, 0, qo:qo + 128], tri[:])
                        pt = pt[:, 0, :]
                    if ui == 1 and pending_norm is not None:
                        pending_norm()
                        pending_norm = None
                    if prev is not None:
                        emit_pv(prev[0], prev[1], prev[2], False)
                    prev = (u, pt, ui == 0)
                emit_pv(prev[0], prev[1], prev[2], True)
                if nu == 4 and pending_norm is not None:
                    # qc=0 groups are short; make sure norms drain
                    pending_norm()
                    pending_norm = None

                def norm(i=i, qc=qc, ov=ov, sm=sm):
                    rcp = rcpp.tile([1, 512], F32, name="rcp_t")
                    nc.vector.reciprocal(rcp[:], sm[:])
                    bc = bcp.tile([128, 512], F32, name="bc_t")
                    nc.gpsimd.partition_broadcast(bc[:], rcp[:])
                    with nc.allow_low_precision(reason="bf16 attn out"):
                        nc.vector.tensor_mul(
                            at[:, i, 512 * qc:512 * (qc + 1)], ov[:], bc[:])

                pending_norm = norm
                if qc > 0 and i == 0:
                    emit_p3(qc - 1)

        pending_norm()
        emit_p3(N_SC - 1)


_NC_CACHE = None


def get_nc():
    global _NC_CACHE
    if _NC_CACHE is not None:
        return _NC_CACHE
    nc = bacc.Bacc("TRN2", target_bir_lowering=False, debug=False,
                   num_devices=8)
    # p-major packed layouts (one DMA per stream per chunk)
    xT = nc.dram_tensor("xT", [128, N_DT, N_SC, 512], BF16,
                        kind="ExternalInput").ap()
    x8 = nc.dram_tensor("x8", [128, N_DT, N_SC, 512], FP8,
                        kind="ExternalInput").ap()
    wq8 = nc.dram_tensor("wq8", [128, N_DT, 512], FP8,
                         kind="ExternalInput").ap()
    wk8 = nc.dram_tensor("wk8", [128, N_DT, 128], FP8,
                         kind="ExternalInput").ap()
    wvT = nc.dram_tensor("wvT", [128, N_DT, 128], BF16,
                         kind="ExternalInput").ap()
    woT = nc.dram_tensor("woT", [128, HPC, S], BF16,
                         kind="ExternalInput").ap()
    ropeA = nc.dram_tensor("ropeA", [128, HPC, N_SC, 512], BF16,
                           kind="ExternalInput").ap()
    ropeBs = nc.dram_tensor("ropeBs", [128, HPC, N_SC, 512], BF16,
                            kind="ExternalInput").ap()
    trid = nc.dram_tensor("trid", [128, 128], BF16, kind="ExternalInput").ap()
    pswap = nc.dram_tensor("pswap", [128, 128], BF16,
                           kind="ExternalInput").ap()
    onesd = nc.dram_tensor("onesd", [128, 1], BF16, kind="ExternalInput").ap()
    outT = nc.dram_tensor("outT", [128, N_SB, S], BF16,
                          kind="ExternalOutput").ap()

    with tile.TileContext(nc) as tc, ExitStack() as ctx:
        build_kernel_body(ctx, tc, outT,
                          (xT, x8, wq8, wk8, wvT, woT, ropeA, ropeBs, trid,
                           pswap, onesd))
    nc.compile()
    _NC_CACHE = nc
    return nc


def host_inputs(x, Wq, Wk, Wv, Wo):
    """Per-core input dicts (core c = 4*b + g)."""
    bf16 = ml_dtypes.bfloat16
    f8 = ml_dtypes.float8_e4m3
    x = np.asarray(x, np.float32)
    Wq = np.asarray(Wq, np.float32)
    Wk = np.asarray(Wk, np.float32)
    Wv = np.asarray(Wv, np.float32)
    Wo = np.asarray(Wo, np.float32)

    freqs = 1.0 / (ROPE_THETA ** (np.arange(0, D, 2, dtype=np.float32) / D))
    ang = np.arange(S, dtype=np.float32)[:, None] * freqs[None, :]  # [S, D/2]
    cos = np.cos(ang).astype(np.float32)
    sin = np.sin(ang).astype(np.float32)

    idx = np.arange(128)
    tri = (idx[:, None] <= idx[None, :]).astype(bf16)  # [kp, t] valid
    pswap = np.zeros((128, 128), np.float32)
    pswap[idx, idx ^ 1] = 1.0
    sgn = np.where(idx % 2 == 0, -1.0, 1.0).astype(np.float32)

    def pack_x(a):  # [D, S] -> [128, N_DT, N_SC, 512]
        return np.ascontiguousarray(
            a.reshape(N_DT, 128, N_SC, 512).transpose(1, 0, 2, 3))

    def pack_w(a):  # [D, cols] -> [128, N_DT, cols]
        return np.ascontiguousarray(
            a.reshape(N_DT, 128, -1).transpose(1, 0, 2))

    def pack_r(a):  # [512, S] -> [128, HPC, N_SC, 512]
        return np.ascontiguousarray(
            a.reshape(HPC, 128, N_SC, 512).transpose(1, 0, 2, 3))

    xTb = [np.ascontiguousarray(x[b].T) for b in range(B)]
    xp = [pack_x(xb) for xb in xTb]

    in_maps = []
    for c in range(8):
        b, g = divmod(c, 4)
        ra = np.empty((512, S), np.float32)
        rbs = np.empty((512, S), np.float32)
        for i in range(HPC):
            fidx = 256 * g + 64 * i + (idx // 2)
            ra[128 * i:128 * (i + 1)] = cos[:, fidx].T
            rb_i = (sin[:, fidx] * sgn[None, :]).T  # [128, S]
            rbs[128 * i:128 * (i + 1)] = rb_i[idx ^ 1]
        in_maps.append({
            "xT": xp[b].astype(bf16),
            "x8": xp[b].astype(f8),
            "wq8": pack_w(
                np.ascontiguousarray(
                    (W8SCALE * Wq[512 * g:512 * (g + 1)]).T)).astype(f8),
            "wk8": pack_w(
                np.ascontiguousarray(
                    (W8SCALE * Wk[128 * g:128 * (g + 1)]).T)).astype(f8),
            "wvT": pack_w(
                np.ascontiguousarray(
                    Wv[128 * g:128 * (g + 1)].T)).astype(bf16),
            "woT": pack_r(
                np.ascontiguousarray(
                    Wo[:, 512 * g:512 * (g + 1)].T)).reshape(
                        128, HPC, S).astype(bf16),
            "ropeA": pack_r(ra).astype(bf16),
            "ropeBs": pack_r(rbs).astype(bf16),
            "trid": tri,
            "pswap": pswap.astype(bf16),
            "onesd": np.ones((128, 1), bf16),
        })
    return in_maps


def kernel(x, Wq, Wk, Wv, Wo, mask, _trace=False):
    in_maps = host_inputs(x, Wq, Wk, Wv, Wo)
    nc = get_nc()
    res = run_bass_kernel_spmd(nc, in_maps, list(range(8)), trace=_trace)
    # outT is [128, N_SB, S] p-major: row 128*jb+p of the [D, S] partial
    # is outT[p, jb, :]
    outs = [np.asarray(res.results[c]["outT"], np.float32)
            .transpose(1, 0, 2).reshape(D, S) for c in range(8)]
    out = np.stack([
        (outs[4 * b + 0] + outs[4 * b + 1] + outs[4 * b + 2]
         + outs[4 * b + 3]).T
        for b in range(B)
    ]).astype(np.float32)
    if _trace:
        kernel.last_result = res
    return out
